# revision 1
# baseline (speedup 1.0000x reference)
"""Trainium2 Bass kernel for CustomYOLOLoss (N=512, S=52, NB=3), 8-core data parallel.

Baseline restore copy (known-good, 234us). See git-less backup note.
"""

import os
import numpy as np

import concourse.bass as bass
import concourse.bacc as bacc
import concourse.mybir as mybir
import concourse.tile as tile
from concourse.bass_utils import run_bass_kernel_spmd

F32 = mybir.dt.float32
U8 = mybir.dt.uint8
AF = mybir.ActivationFunctionType
ALU = mybir.AluOpType

N, S, NB = 512, 52, 3
CORES = 8
NPC = N // CORES
P = 128
CELLS = NPC * S * S
X = CELLS // P
EPS = 1e-7

F = int(os.environ.get("YOLO_F", "169"))
REPEAT = int(os.environ.get("YOLO_REPEAT", "1"))
NBLK = X // F
NACC = 6

_nc_cache = {}


def build_nc():
    key = (F, REPEAT)
    if key in _nc_cache:
        return _nc_cache[key]
    nc = bacc.Bacc(trn_type="TRN2", target_bir_lowering=False)
    inp = nc.dram_tensor("input", [P, X * 15], F32, kind="ExternalInput")
    tgt = nc.dram_tensor("target", [P, X * 5], F32, kind="ExternalInput")
    out = nc.dram_tensor("out", [P, NACC * NBLK], F32, kind="ExternalOutput")

    with tile.TileContext(nc) as tc:
        with (
            tc.tile_pool(name="dma", bufs=1) as dma_pool,
            tc.tile_pool(name="work", bufs=1) as work,
            tc.tile_pool(name="scr", bufs=2) as scr,
            tc.tile_pool(name="actout", bufs=2) as actout,
            tc.tile_pool(name="acts1", bufs=1) as acts1,
            tc.tile_pool(name="accp", bufs=1) as accp,
        ):
            acc = accp.tile([P, NACC * NBLK], F32)

            tins, ttgs = [], []
            for blk in range(NBLK):
                tin = dma_pool.tile([P, F * 15], F32, tag=f"tin{blk}")
                ttg = dma_pool.tile([P, F * 5], F32, tag=f"ttg{blk}")
                nc.sync.dma_start(tin[:], inp[:, blk * F * 15:(blk + 1) * F * 15])
                nc.sync.dma_start(ttg[:], tgt[:, blk * F * 5:(blk + 1) * F * 5])
                tins.append(tin)
                ttgs.append(ttg)

            for blk in range(NBLK):
                tin, ttg = tins[blk], ttgs[blk]
                tin_v = tin[:].rearrange("p (f b c) -> p f b c", b=3, c=5)
                ttg_v = ttg[:].rearrange("p (f c) -> p f c", c=5)
                coords_in = tin_v[:, :, :, 1:5]
                conf_all = tin_v[:, :, :, 0]

                def conf_b(b, tin_v=tin_v):
                    return tin_v[:, :, b, 0]

                sig = actout.tile([P, 12 * F], F32, tag="sig")
                sig_v = sig[:].rearrange("p (b c f) -> p f b c", b=3, c=4)
                nc.scalar.activation(sig_v, coords_in, AF.Sigmoid)
                sig_r = sig[:].rearrange("p (b c f) -> p c b f", b=3, c=4)

                ext = acts1.tile([P, 3 * F], F32, tag=f"ext{blk % 8}")
                ext_v = ext[:].rearrange("p (b f) -> p f b", b=3)
                nc.scalar.activation(ext_v, conf_all, AF.Exp, scale=-1.0)
                bce1 = acts1.tile([P, 3 * F], F32, tag=f"bce1{blk % 8}")
                nc.scalar.activation(bce1[:], ext[:], AF.Ln, bias=1.0)

                t22 = work.tile([P, 2 * F], F32, tag="t22")
                t22_v = t22[:].rearrange("p (c f) -> p f c", c=2)
                nc.vector.tensor_scalar(t22_v, ttg_v[:, :, 3:5], 0.5, None, ALU.mult)
                areab = work.tile([P, F], F32, tag="areab")
                nc.vector.tensor_tensor(areab[:], ttg_v[:, :, 3], ttg_v[:, :, 4],
                                        ALU.mult)
                nc.vector.tensor_scalar(areab[:], areab[:], EPS, None, ALU.add)
                obj = work.tile([P, F], F32, tag="obj")
                nc.vector.tensor_scalar(obj[:], ttg_v[:, :, 0], 0.0, None, ALU.is_gt)

                p22 = work.tile([P, 6 * F], F32, tag="p22")
                p22_whv = p22[:].rearrange("p (b c f) -> p c b f", b=3, c=2)
                nc.vector.tensor_scalar(p22_whv, sig_r[:, 2:4, :, :], 0.5, None,
                                        ALU.mult)
                bxy1 = work.tile([P, 2 * F], F32, tag="bxy1")
                bxy1_v = bxy1[:].rearrange("p (c f) -> p f c", c=2)
                nc.vector.tensor_tensor(bxy1_v, ttg_v[:, :, 1:3],
                                        t22[:].rearrange("p (c f) -> p f c", c=2),
                                        ALU.subtract)
                bxy2 = work.tile([P, 2 * F], F32, tag="bxy2")
                bxy2_v = bxy2[:].rearrange("p (c f) -> p f c", c=2)
                nc.vector.tensor_tensor(bxy2_v, ttg_v[:, :, 1:3],
                                        t22[:].rearrange("p (c f) -> p f c", c=2),
                                        ALU.add)
                axy1 = work.tile([P, 6 * F], F32, tag="axy1")
                axy2 = work.tile([P, 6 * F], F32, tag="axy2")
                for b in range(NB):
                    sxy = sig[:, (4 * b) * F:(4 * b + 2) * F]
                    pb_ = p22[:, b * 2 * F:(b + 1) * 2 * F]
                    nc.vector.tensor_tensor(axy1[:, b * 2 * F:(b + 1) * 2 * F],
                                            sxy, pb_, ALU.subtract)
                    nc.vector.tensor_tensor(axy2[:, b * 2 * F:(b + 1) * 2 * F],
                                            sxy, pb_, ALU.add)
                lt = work.tile([P, 6 * F], F32, tag="lt")
                elt = work.tile([P, 6 * F], F32, tag="elt")
                for b in range(NB):
                    sl6 = slice(b * 2 * F, (b + 1) * 2 * F)
                    nc.vector.tensor_tensor(lt[:, sl6], axy1[:, sl6], bxy1[:],
                                            ALU.max)
                    nc.vector.tensor_tensor(elt[:, sl6], axy1[:, sl6], bxy1[:],
                                            ALU.min)
                    nc.vector.tensor_tensor(axy1[:, sl6], axy2[:, sl6], bxy2[:],
                                            ALU.min)
                    nc.vector.tensor_tensor(axy2[:, sl6], axy2[:, sl6], bxy2[:],
                                            ALU.max)
                iwr = work.tile([P, 6 * F], F32, tag="iwr")
                nc.vector.tensor_tensor(iwr[:], axy1[:], lt[:], ALU.subtract)
                nc.vector.tensor_scalar(iwr[:], iwr[:], 0.0, None, ALU.max)
                ew = work.tile([P, 6 * F], F32, tag="ew")
                nc.vector.tensor_tensor(ew[:], axy2[:], elt[:], ALU.subtract)

                geo = work.tile([P, 9 * F], F32, tag="geo")
                iwr_v = iwr[:].rearrange("p (b c f) -> p c b f", b=3, c=2)
                nc.vector.tensor_tensor(
                    geo[:, 0:3 * F].rearrange("p (b f) -> p b f", b=3),
                    iwr_v[:, 0, :, :], iwr_v[:, 1, :, :], ALU.mult)
                ew_v = ew[:].rearrange("p (b c f) -> p c b f", b=3, c=2)
                nc.vector.tensor_tensor(
                    geo[:, 6 * F:9 * F].rearrange("p (b f) -> p b f", b=3),
                    ew_v[:, 0, :, :], ew_v[:, 1, :, :], ALU.mult)
                aa = work.tile([P, 3 * F], F32, tag="aa")
                nc.vector.tensor_tensor(
                    aa[:].rearrange("p (b f) -> p b f", b=3),
                    sig_r[:, 2, :, :], sig_r[:, 3, :, :], ALU.mult)
                for b in range(NB):
                    nc.vector.tensor_tensor(geo[:, (3 + b) * F:(4 + b) * F],
                                            aa[:, b * F:(b + 1) * F], areab[:],
                                            ALU.add)
                nc.vector.tensor_tensor(geo[:, 3 * F:6 * F], geo[:, 3 * F:6 * F],
                                        geo[:, 0:3 * F], ALU.subtract)
                rue = work.tile([P, 3 * F], F32, tag="rue")
                nc.vector.reciprocal_approx_fast(rue[:], geo[:, 3 * F:6 * F])
                nc.vector.tensor_tensor(geo[:, 0:3 * F], geo[:, 0:3 * F], rue[:],
                                        ALU.mult)

                iou0, iou1, iou2 = (geo[:, b * F:(b + 1) * F] for b in range(3))
                mk = work.tile([P, 2 * F], U8, tag="mk")
                mxt = work.tile([P, F], F32, tag="mxt")
                nc.vector.tensor_tensor(mk[:, 0:F], iou1, iou0, ALU.is_gt)
                nc.vector.tensor_tensor(mxt[:], iou0, iou1, ALU.max)
                nc.vector.tensor_tensor(mk[:, F:2 * F], iou2, mxt[:], ALU.is_gt)
                sel3 = work.tile([P, 3 * F], F32, tag="sel3")
                for q in range(3):
                    sq = sel3[:, q * F:(q + 1) * F]
                    base = q * 3 * F
                    nc.vector.tensor_copy(sq, geo[:, base:base + F])
                    nc.vector.copy_predicated(sq, mk[:, 0:F],
                                              geo[:, base + F:base + 2 * F])
                    nc.vector.copy_predicated(sq, mk[:, F:2 * F],
                                              geo[:, base + 2 * F:base + 3 * F])

                ee = work.tile([P, F], F32, tag="ee")
                nc.vector.tensor_scalar(ee[:], sel3[:, 2 * F:3 * F], EPS, None,
                                        ALU.add)
                dd = work.tile([P, F], F32, tag="dd")
                nc.vector.tensor_tensor(dd[:], ee[:], sel3[:, F:2 * F],
                                        ALU.subtract)
                ree = work.tile([P, F], F32, tag="ree")
                nc.vector.reciprocal_approx_fast(ree[:], ee[:])
                tt = work.tile([P, F], F32, tag="tt")
                nc.vector.tensor_tensor(tt[:], dd[:], ree[:], ALU.mult)
                nc.vector.tensor_tensor(dd[:], sel3[:, 0:F], tt[:], ALU.subtract)

                csel = work.tile([P, F], F32, tag="csel")
                nc.vector.tensor_copy(csel[:], conf_b(0))
                nc.vector.copy_predicated(csel[:], mk[:, 0:F], conf_b(1))
                nc.vector.copy_predicated(csel[:], mk[:, F:2 * F], conf_b(2))
                bsel = work.tile([P, F], F32, tag="bsel")
                nc.vector.tensor_copy(bsel[:], bce1[:, 0:F])
                nc.vector.copy_predicated(bsel[:], mk[:, 0:F], bce1[:, F:2 * F])
                nc.vector.copy_predicated(bsel[:], mk[:, F:2 * F],
                                          bce1[:, 2 * F:3 * F])

                cs = work.tile([P, F], F32, tag="cs")
                nc.vector.tensor_tensor(cs[:], conf_b(0), conf_b(1), ALU.add)
                nc.vector.tensor_tensor(cs[:], cs[:], conf_b(2), ALU.add)
                bs = work.tile([P, F], F32, tag="bs")
                nc.vector.tensor_tensor(bs[:], bce1[:, 0:F], bce1[:, F:2 * F],
                                        ALU.add)
                nc.vector.tensor_tensor(bs[:], bs[:], bce1[:, 2 * F:3 * F],
                                        ALU.add)
                nc.vector.tensor_tensor(cs[:], cs[:], bs[:], ALU.add)

                col = blk * NACC

                def acccol(i, col=col):
                    return acc[:, col + i:col + i + 1]

                nc.vector.tensor_reduce(acccol(0), cs[:], mybir.AxisListType.X,
                                        ALU.add)
                for i, val in ((1, cs[:]), (2, csel[:]), (3, bsel[:]), (4, dd[:])):
                    sc = scr.tile([P, F], F32, tag="ttr_scr")
                    nc.vector.tensor_tensor(sc[:], obj[:], val, ALU.mult)
                    nc.scalar.activation(sc[:], sc[:], AF.Copy,
                                         accum_out=acccol(i))
                nc.vector.tensor_reduce(acccol(5), obj[:], mybir.AxisListType.X,
                                        ALU.add)

            nc.gpsimd.dma_start(out[:], acc[:])

    nc.compile()
    _nc_cache[key] = nc
    return nc


def kernel(input, target):
    nc = build_nc()
    in_maps = []
    for c in range(CORES):
        sl = slice(c * NPC, (c + 1) * NPC)
        in_maps.append({
            "input": np.ascontiguousarray(input[sl]).reshape(P, X * 15),
            "target": np.ascontiguousarray(target[sl]).reshape(P, X * 5),
        })
    res = run_bass_kernel_spmd(nc, in_maps, core_ids=list(range(CORES)))
    total = np.zeros(NACC, dtype=np.float64)
    for r in res.results:
        total += r["out"].reshape(P, NBLK, NACC).sum(axis=(0, 1), dtype=np.float64)
    S_all, T1, T2, NO, G, NOBJ = total
    n_obj = NOBJ
    n_noobj = float(N * S * S) - n_obj
    num1 = S_all - T1
    num2 = T1 - T2 - NO
    num_bbox = n_obj - G
    loss_noobj = num1 / (n_noobj * NB) + num2 / (n_obj * (NB - 1))
    loss_bbox = num_bbox / n_obj
    loss_obj = NO / n_obj
    loss = loss_obj + loss_bbox + loss_noobj
    return (np.float32(loss), np.float32(loss_noobj), np.float32(loss_bbox),
            np.float32(loss_obj))



# revision 11
# speedup vs baseline: 1.7188x; 1.7188x over previous
"""Trainium2 Bass kernel for CustomYOLOLoss (N=512, S=52, NB=3), 8-core data parallel.

SoA bf16 redesign:
  - Host: cast inputs to bf16 and repack channel-major (15 resp. 5 planes of
    [128, 1352] cells per core). Contiguous unit-stride planes let the DVE run
    tensor_tensor in 2x_1p mode and halve HBM traffic.
  - Geometry per box uses the identity
      inter_w = (wa/2 + wb/2) - max(|xa-xb|, |wa/2 - wb/2|)   (clipped at 0)
      enc_w   = (wa/2 + wb/2) + max(|xa-xb|, |wa/2 - wb/2|)
    with a single abs_max ALU op.
  - BCE via a single Softplus activation (bce0 = softplus(conf));
    bce1_resp = softplus(c_resp) - c_resp recovered on the host from sums.
  - Responsible-box argmax via cross-multiplication (inter_i * union_j
    comparisons) -> only 2 divisions per cell-chunk (after selection).
  - Masked sums fused into tensor_tensor_reduce accumulators; host combines
    the 6 partial sums (A0, T1, S0R, CR, G, NOBJ) into the 4 loss terms.
"""

import numpy as np
import ml_dtypes

import concourse.bass as bass
import concourse.bacc as bacc
import concourse.mybir as mybir
import concourse.tile as tile
from concourse.bass_utils import run_bass_kernel_spmd

F32 = mybir.dt.float32
BF16 = mybir.dt.bfloat16
AF = mybir.ActivationFunctionType
ALU = mybir.AluOpType

N, S, NB = 512, 52, 3
CORES = 8
NPC = N // CORES          # 64 images per core
P = 128
CELLS = NPC * S * S       # 173056
X = CELLS // P            # 1352 cells per partition
W = 676                   # chunk width (free dim)
NCH = X // W              # 2 chunks
NACC = 6                  # A0, T1, S0R, CR, G, NOBJ

# input channel c = b*5 + k (k=0 conf, 1..4 box) -> plane order
# [x0 x1 x2  y0 y1 y2  w0 w1 w2  h0 h1 h2  c0 c1 c2]
PERM_IN = [1, 6, 11, 2, 7, 12, 3, 8, 13, 4, 9, 14, 0, 5, 10]
# target channel order -> [TX TY TW TH TC]
PERM_TG = [1, 2, 3, 4, 0]

_nc_cache = {}


def build_nc():
    if "nc" in _nc_cache:
        return _nc_cache["nc"]
    nc = bacc.Bacc(trn_type="TRN2", target_bir_lowering=False)
    inp = nc.dram_tensor("input", [P, 15 * X], BF16, kind="ExternalInput")
    tgt = nc.dram_tensor("target", [P, 5 * X], BF16, kind="ExternalInput")
    out = nc.dram_tensor("out", [P, NACC * NCH], F32, kind="ExternalOutput")

    inp_v = inp[:].rearrange("p (c x) -> p c x", c=15)
    tgt_v = tgt[:].rearrange("p (c x) -> p c x", c=5)

    with tile.TileContext(nc) as tc:
        with (
            tc.tile_pool(name="dma", bufs=1) as dma_pool,
            tc.tile_pool(name="big", bufs=1) as big,
            tc.tile_pool(name="work", bufs=1) as work,
            tc.tile_pool(name="accp", bufs=1) as accp,
        ):
            acc = accp.tile([P, NACC * NCH], F32)

            # ---- DMA all inputs up front (distinct tiles, deep queue) ----
            boxes, tgts = [], []
            for ch in range(NCH):
                sl = slice(ch * W, (ch + 1) * W)
                box = dma_pool.tile([P, 12 * W], BF16, tag=f"box{ch}")
                box_v = box[:].rearrange("p (c x) -> p c x", c=12)
                nc.sync.dma_start(box_v, inp_v[:, 0:12, sl])
                tg = dma_pool.tile([P, 5 * W], BF16, tag=f"tgt{ch}")
                tg_v = tg[:].rearrange("p (c x) -> p c x", c=5)
                nc.sync.dma_start(tg_v, tgt_v[:, :, sl])
                boxes.append(box)
                tgts.append(tg)
            conf = dma_pool.tile([P, 3 * X], BF16, tag="conf")
            conf_v = conf[:].rearrange("p (c x) -> p c x", c=3)
            nc.sync.dma_start(conf_v, inp_v[:, 12:15, :])

            # ---- scalar engine: sigmoids (in place), then exp+ln ----
            # bce1_b = softplus(-c_b) = ln(1 + exp(-c_b))
            for ch in range(NCH):
                nc.scalar.activation(boxes[ch][:], boxes[ch][:], AF.Sigmoid)
            s0 = big.tile([P, 3 * X], BF16, tag="s0")
            nc.scalar.activation(s0[:], conf[:], AF.Exp, scale=-1.0)
            nc.scalar.activation(s0[:], s0[:], AF.Ln, bias=1.0)

            for ch in range(NCH):
                box, tg = boxes[ch], tgts[ch]
                TX = tg[:, 0 * W:1 * W]
                TY = tg[:, 1 * W:2 * W]
                TWp = tg[:, 2 * W:3 * W]
                THp = tg[:, 3 * W:4 * W]
                TC = tg[:, 4 * W:5 * W]
                Xg = box[:, 0 * W:3 * W]
                Yg = box[:, 3 * W:6 * W]
                Wg = box[:, 6 * W:9 * W]
                Hg = box[:, 9 * W:12 * W]

                tw2 = work.tile([P, 2 * W], BF16, tag="tw2")
                nc.vector.tensor_scalar(tw2[:, 0:W], TWp, 0.5, None, ALU.mult)
                nc.vector.tensor_scalar(tw2[:, W:2 * W], THp, 0.5, None,
                                        ALU.mult)

                d_all = work.tile([P, 6 * W], BF16, tag="d_all")
                for b in range(3):
                    nc.vector.tensor_tensor(d_all[:, b * W:(b + 1) * W],
                                            Xg[:, b * W:(b + 1) * W], TX,
                                            ALU.subtract)
                    nc.vector.tensor_tensor(d_all[:, (3 + b) * W:(4 + b) * W],
                                            Yg[:, b * W:(b + 1) * W], TY,
                                            ALU.subtract)
                dw = work.tile([P, 6 * W], BF16, tag="dw")
                sw = work.tile([P, 6 * W], BF16, tag="sw")
                for b in range(3):
                    nc.vector.scalar_tensor_tensor(
                        dw[:, b * W:(b + 1) * W], Wg[:, b * W:(b + 1) * W],
                        0.5, tw2[:, 0:W], ALU.mult, ALU.subtract)
                    nc.vector.scalar_tensor_tensor(
                        dw[:, (3 + b) * W:(4 + b) * W],
                        Hg[:, b * W:(b + 1) * W],
                        0.5, tw2[:, W:2 * W], ALU.mult, ALU.subtract)
                    nc.vector.scalar_tensor_tensor(
                        sw[:, b * W:(b + 1) * W], Wg[:, b * W:(b + 1) * W],
                        0.5, tw2[:, 0:W], ALU.mult, ALU.add)
                    nc.vector.scalar_tensor_tensor(
                        sw[:, (3 + b) * W:(4 + b) * W],
                        Hg[:, b * W:(b + 1) * W],
                        0.5, tw2[:, W:2 * W], ALU.mult, ALU.add)
                # m = max(|d|, |dw|) = max(max(d,dw), -min(d,dw))
                m_t = work.tile([P, 6 * W], BF16, tag="m_t")
                nc.vector.tensor_tensor(m_t[:], d_all[:], dw[:], ALU.max)
                nc.vector.tensor_tensor(dw[:], d_all[:], dw[:], ALU.min)
                nc.vector.scalar_tensor_tensor(m_t[:], dw[:], -1.0, m_t[:],
                                               ALU.mult, ALU.max)
                iw = work.tile([P, 6 * W], BF16, tag="iw")
                nc.vector.tensor_tensor(iw[:], sw[:], m_t[:], ALU.subtract)
                nc.vector.tensor_scalar(iw[:], iw[:], 0.0, None, ALU.max)
                # ew = sw + m  (in place over sw)
                nc.vector.tensor_tensor(sw[:], sw[:], m_t[:], ALU.add)

                inter = work.tile([P, 3 * W], BF16, tag="inter")
                nc.vector.tensor_tensor(inter[:], iw[:, 0:3 * W],
                                        iw[:, 3 * W:6 * W], ALU.mult)
                enc = work.tile([P, 3 * W], BF16, tag="enc")
                nc.vector.tensor_tensor(enc[:], sw[:, 0:3 * W],
                                        sw[:, 3 * W:6 * W], ALU.mult)
                union = work.tile([P, 3 * W], BF16, tag="union")
                nc.vector.tensor_tensor(union[:], Wg, Hg, ALU.mult)
                areab3 = work.tile([P, 3 * W], BF16, tag="areab3")
                nc.vector.tensor_tensor(areab3[:, 0:W], TWp, THp, ALU.mult)
                nc.vector.tensor_copy(areab3[:, W:2 * W], areab3[:, 0:W])
                nc.vector.tensor_copy(areab3[:, 2 * W:3 * W], areab3[:, 0:W])
                nc.vector.tensor_tensor(union[:], union[:], areab3[:], ALU.add)
                nc.vector.tensor_tensor(union[:], union[:], inter[:],
                                        ALU.subtract)

                # responsible box via cross-multiplied IoU comparisons
                pq = work.tile([P, 2 * W], BF16, tag="pq")
                mk = work.tile([P, 2 * W], mybir.dt.uint8, tag="mk")
                i0, i1, i2 = (inter[:, b * W:(b + 1) * W] for b in range(3))
                u0, u1, u2 = (union[:, b * W:(b + 1) * W] for b in range(3))
                e0, e1, e2 = (enc[:, b * W:(b + 1) * W] for b in range(3))
                nc.vector.tensor_tensor(pq[:, 0:W], i1, u0, ALU.mult)
                nc.vector.tensor_tensor(pq[:, W:2 * W], i0, u1, ALU.mult)
                nc.vector.tensor_tensor(mk[:, 0:W], pq[:, 0:W], pq[:, W:2 * W],
                                        ALU.is_gt)
                mk1, mk2 = mk[:, 0:W], mk[:, W:2 * W]
                nc.vector.copy_predicated(i0, mk1, i1)
                nc.vector.copy_predicated(u0, mk1, u1)
                nc.vector.tensor_tensor(pq[:, 0:W], i2, u0, ALU.mult)
                nc.vector.tensor_tensor(pq[:, W:2 * W], i0, u2, ALU.mult)
                nc.vector.tensor_tensor(mk2, pq[:, 0:W], pq[:, W:2 * W],
                                        ALU.is_gt)
                nc.vector.copy_predicated(i0, mk2, i2)
                nc.vector.copy_predicated(u0, mk2, u2)
                nc.vector.copy_predicated(e0, mk1, e1)
                nc.vector.copy_predicated(e0, mk2, e2)

                # g = iou + union/enc  (= giou + 1)
                f32t = work.tile([P, 4 * W], F32, tag="f32t")
                uf, ef = f32t[:, 0:W], f32t[:, W:2 * W]
                ru, re = f32t[:, 2 * W:3 * W], f32t[:, 3 * W:4 * W]
                nc.vector.tensor_copy(uf, u0)
                nc.vector.tensor_copy(ef, e0)
                nc.vector.reciprocal_approx_fast(ru, uf)
                nc.vector.reciprocal_approx_fast(re, ef)
                gio = work.tile([P, 3 * W], BF16, tag="gio")
                iou, t_, g = (gio[:, b * W:(b + 1) * W] for b in range(3))
                nc.vector.tensor_tensor(iou, i0, ru, ALU.mult)
                nc.vector.tensor_tensor(t_, uf, re, ALU.mult)
                nc.vector.tensor_tensor(g, iou, t_, ALU.add)

                # conf sums and responsible-box selections
                c0 = conf[:, 0 * X + ch * W:0 * X + (ch + 1) * W]
                c1 = conf[:, 1 * X + ch * W:1 * X + (ch + 1) * W]
                c2 = conf[:, 2 * X + ch * W:2 * X + (ch + 1) * W]
                s0_0 = s0[:, 0 * X + ch * W:0 * X + (ch + 1) * W]
                s0_1 = s0[:, 1 * X + ch * W:1 * X + (ch + 1) * W]
                s0_2 = s0[:, 2 * X + ch * W:2 * X + (ch + 1) * W]

                def col(i, ch=ch):
                    j = ch * NACC + i
                    return acc[:, j:j + 1]

                # q1 = sum_b bce0_b = (c0+c1+c2) + (b0+b1+b2)
                q1 = work.tile([P, 3 * W], BF16, tag="q1")
                scr = q1[:, 2 * W:3 * W]
                nc.vector.tensor_tensor(q1[:, 0:W], c0, c1, ALU.add)
                nc.vector.tensor_tensor(q1[:, 0:W], q1[:, 0:W], c2, ALU.add)
                nc.vector.tensor_tensor(q1[:, W:2 * W], s0_0, s0_1, ALU.add)
                nc.vector.tensor_tensor(q1[:, W:2 * W], q1[:, W:2 * W], s0_2,
                                        ALU.add)
                nc.vector.scalar_tensor_tensor(q1[:, 0:W], q1[:, 0:W], 0.0,
                                               q1[:, W:2 * W], ALU.add,
                                               ALU.add, accum_out=col(0))
                nc.vector.copy_predicated(c0, mk1, c1)
                nc.vector.copy_predicated(c0, mk2, c2)
                nc.vector.copy_predicated(s0_0, mk1, s0_1)
                nc.vector.copy_predicated(s0_0, mk2, s0_2)
                nc.vector.scalar_tensor_tensor(scr, q1[:, 0:W], 0.0, TC,
                                               ALU.add, ALU.mult,
                                               accum_out=col(1))
                nc.vector.scalar_tensor_tensor(scr, s0_0, 0.0, TC,
                                               ALU.add, ALU.mult,
                                               accum_out=col(2))
                nc.vector.scalar_tensor_tensor(scr, c0, 0.0, TC,
                                               ALU.add, ALU.mult,
                                               accum_out=col(3))
                nc.vector.scalar_tensor_tensor(scr, g, 0.0, TC,
                                               ALU.add, ALU.mult,
                                               accum_out=col(4))
                nc.vector.tensor_reduce(col(5), TC, mybir.AxisListType.X,
                                        ALU.add)

            nc.gpsimd.dma_start(out[:], acc[:])

    nc.compile()
    _nc_cache["nc"] = nc
    return nc


def make_in_maps(input, target):
    in_maps = []
    for c in range(CORES):
        sl = slice(c * NPC, (c + 1) * NPC)
        a = input[sl].reshape(P, X, 15)[:, :, PERM_IN].transpose(0, 2, 1)
        b = target[sl].reshape(P, X, 5)[:, :, PERM_TG].transpose(0, 2, 1)
        in_maps.append({
            "input": np.ascontiguousarray(a).astype(ml_dtypes.bfloat16).reshape(
                P, 15 * X),
            "target": np.ascontiguousarray(b).astype(ml_dtypes.bfloat16).reshape(
                P, 5 * X),
        })
    return in_maps


def kernel(input, target):
    nc = build_nc()
    in_maps = make_in_maps(input, target)
    res = run_bass_kernel_spmd(nc, in_maps, core_ids=list(range(CORES)))
    total = np.zeros(NACC, dtype=np.float64)
    for r in res.results:
        total += r["out"].reshape(P, NCH, NACC).sum(axis=(0, 1),
                                                    dtype=np.float64)
    A0, T1, NO, CR, G, NOBJ = total
    n_obj = NOBJ
    n_noobj = float(N * S * S) - n_obj
    s0r = NO + CR  # sum obj * softplus(c_resp)
    loss_noobj = (A0 - T1) / (n_noobj * NB) + (T1 - s0r) / (n_obj * (NB - 1))
    loss_obj = NO / n_obj
    loss_bbox = (2.0 * n_obj - G) / n_obj
    loss = loss_obj + loss_bbox + loss_noobj
    return (np.float32(loss), np.float32(loss_noobj), np.float32(loss_bbox),
            np.float32(loss_obj))


# revision 17
# speedup vs baseline: 1.8657x; 1.0855x over previous
"""Trainium2 Bass kernel for CustomYOLOLoss (N=512, S=52, NB=3), 8-core data parallel.

SoA bf16 redesign:
  - Host: cast inputs to bf16 and repack channel-major (15 resp. 5 planes of
    [128, 1352] cells per core). Contiguous unit-stride planes let the DVE run
    tensor_tensor in 2x_1p mode and halve HBM traffic.
  - Geometry per box uses the identity
      inter_w = (wa/2 + wb/2) - max(|xa-xb|, |wa/2 - wb/2|)   (clipped at 0)
      enc_w   = (wa/2 + wb/2) + max(|xa-xb|, |wa/2 - wb/2|)
    with a single abs_max ALU op.
  - BCE via a single Softplus activation (bce0 = softplus(conf));
    bce1_resp = softplus(c_resp) - c_resp recovered on the host from sums.
  - Responsible-box argmax via cross-multiplication (inter_i * union_j
    comparisons) -> only 2 divisions per cell-chunk (after selection).
  - Masked sums fused into tensor_tensor_reduce accumulators; host combines
    the 6 partial sums (A0, T1, S0R, CR, G, NOBJ) into the 4 loss terms.
"""

import numpy as np
import ml_dtypes

import concourse.bass as bass
import concourse.bacc as bacc
import concourse.mybir as mybir
import concourse.tile as tile
from concourse.bass_utils import run_bass_kernel_spmd

F32 = mybir.dt.float32
BF16 = mybir.dt.bfloat16
AF = mybir.ActivationFunctionType
ALU = mybir.AluOpType

N, S, NB = 512, 52, 3
CORES = 8
NPC = N // CORES          # 64 images per core
P = 128
CELLS = NPC * S * S       # 173056
X = CELLS // P            # 1352 cells per partition
W = 676                   # chunk width (free dim)
NCH = X // W              # 2 chunks
NACC = 6                  # A0, T1, S0R, CR, G, NOBJ

# input channel c = b*5 + k (k=0 conf, 1..4 box) -> plane order
# [x0 x1 x2  y0 y1 y2  w0 w1 w2  h0 h1 h2  c0 c1 c2]
PERM_IN = [1, 6, 11, 2, 7, 12, 3, 8, 13, 4, 9, 14, 0, 5, 10]
# target channel order -> [TX TY TW TH TC]
PERM_TG = [1, 2, 3, 4, 0]

_nc_cache = {}


def build_nc():
    if "nc" in _nc_cache:
        return _nc_cache["nc"]
    nc = bacc.Bacc(trn_type="TRN2", target_bir_lowering=False)
    inp = nc.dram_tensor("input", [P, 15 * X], BF16, kind="ExternalInput")
    tgt = nc.dram_tensor("target", [P, 5 * X], BF16, kind="ExternalInput")
    out = nc.dram_tensor("out", [P, NACC * NCH], F32, kind="ExternalOutput")

    inp_v = inp[:].rearrange("p (c x) -> p c x", c=15)
    tgt_v = tgt[:].rearrange("p (c x) -> p c x", c=5)

    with tile.TileContext(nc) as tc:
        with (
            tc.tile_pool(name="dma", bufs=1) as dma_pool,
            tc.tile_pool(name="big", bufs=1) as big,
            tc.tile_pool(name="work", bufs=1) as work,
            tc.tile_pool(name="accp", bufs=1) as accp,
        ):
            acc = accp.tile([P, NACC * NCH], F32)

            # ---- DMA all inputs up front (distinct tiles, deep queue) ----
            boxes, tgts = [], []
            for ch in range(NCH):
                sl = slice(ch * W, (ch + 1) * W)
                box = dma_pool.tile([P, 12 * W], BF16, tag=f"box{ch}")
                box_v = box[:].rearrange("p (c x) -> p c x", c=12)
                nc.sync.dma_start(box_v, inp_v[:, 0:12, sl])
                tg = dma_pool.tile([P, 5 * W], BF16, tag=f"tgt{ch}")
                tg_v = tg[:].rearrange("p (c x) -> p c x", c=5)
                nc.sync.dma_start(tg_v, tgt_v[:, :, sl])
                boxes.append(box)
                tgts.append(tg)
            conf = dma_pool.tile([P, 3 * X], BF16, tag="conf")
            conf_v = conf[:].rearrange("p (c x) -> p c x", c=3)
            nc.sync.dma_start(conf_v, inp_v[:, 12:15, :])

            # ---- scalar engine: sigmoids (in place), then exp+ln ----
            # bce1_b = softplus(-c_b) = ln(1 + exp(-c_b))
            for ch in range(NCH):
                nc.scalar.activation(boxes[ch][:], boxes[ch][:], AF.Sigmoid)
            s0 = big.tile([P, 3 * X], BF16, tag="s0")
            nc.scalar.activation(s0[:], conf[:], AF.Exp, scale=-1.0)
            nc.scalar.activation(s0[:], s0[:], AF.Ln, bias=1.0)

            for ch in range(NCH):
                box, tg = boxes[ch], tgts[ch]
                TX = tg[:, 0 * W:1 * W]
                TY = tg[:, 1 * W:2 * W]
                TWp = tg[:, 2 * W:3 * W]
                THp = tg[:, 3 * W:4 * W]
                TC = tg[:, 4 * W:5 * W]
                Xg = box[:, 0 * W:3 * W]
                Yg = box[:, 3 * W:6 * W]
                Wg = box[:, 6 * W:9 * W]
                Hg = box[:, 9 * W:12 * W]

                tw2 = work.tile([P, 2 * W], BF16, tag="tw2")
                nc.vector.tensor_scalar(tw2[:, 0:W], TWp, 0.5, None, ALU.mult)
                nc.vector.tensor_scalar(tw2[:, W:2 * W], THp, 0.5, None,
                                        ALU.mult)

                d_all = work.tile([P, 6 * W], BF16, tag="d_all")
                for b in range(3):
                    nc.vector.tensor_tensor(d_all[:, b * W:(b + 1) * W],
                                            Xg[:, b * W:(b + 1) * W], TX,
                                            ALU.subtract)
                    nc.vector.tensor_tensor(d_all[:, (3 + b) * W:(4 + b) * W],
                                            Yg[:, b * W:(b + 1) * W], TY,
                                            ALU.subtract)
                # w2 = 0.5 * [w planes | h planes]  (single-src TS, 4x mode)
                w2 = work.tile([P, 6 * W], BF16, tag="w2")
                nc.vector.tensor_scalar(w2[:], box[:, 6 * W:12 * W], 0.5,
                                        None, ALU.mult)
                dw = work.tile([P, 6 * W], BF16, tag="dw")
                sw = work.tile([P, 6 * W], BF16, tag="sw")
                for b in range(3):
                    wsl = slice(b * W, (b + 1) * W)
                    hsl = slice((3 + b) * W, (4 + b) * W)
                    nc.vector.tensor_tensor(dw[:, wsl], w2[:, wsl],
                                            tw2[:, 0:W], ALU.subtract)
                    nc.vector.tensor_tensor(dw[:, hsl], w2[:, hsl],
                                            tw2[:, W:2 * W], ALU.subtract)
                    nc.vector.tensor_tensor(sw[:, wsl], w2[:, wsl],
                                            tw2[:, 0:W], ALU.add)
                    nc.vector.tensor_tensor(sw[:, hsl], w2[:, hsl],
                                            tw2[:, W:2 * W], ALU.add)
                # m = max(|d|, |dw|) = max(max(d,dw), -min(d,dw))
                m_t = work.tile([P, 6 * W], BF16, tag="m_t")
                nc.vector.tensor_tensor(m_t[:], d_all[:], dw[:], ALU.max)
                nc.vector.tensor_tensor(dw[:], d_all[:], dw[:], ALU.min)
                nc.vector.tensor_scalar(dw[:], dw[:], -1.0, None, ALU.mult)
                nc.vector.tensor_tensor(m_t[:], dw[:], m_t[:], ALU.max)
                iw = work.tile([P, 6 * W], BF16, tag="iw")
                nc.vector.tensor_tensor(iw[:], sw[:], m_t[:], ALU.subtract)
                nc.vector.tensor_scalar(iw[:], iw[:], 0.0, None, ALU.max)
                # ew = sw + m  (in place over sw)
                nc.vector.tensor_tensor(sw[:], sw[:], m_t[:], ALU.add)

                inter = work.tile([P, 3 * W], BF16, tag="inter")
                nc.vector.tensor_tensor(inter[:], iw[:, 0:3 * W],
                                        iw[:, 3 * W:6 * W], ALU.mult)
                enc = work.tile([P, 3 * W], BF16, tag="enc")
                nc.vector.tensor_tensor(enc[:], sw[:, 0:3 * W],
                                        sw[:, 3 * W:6 * W], ALU.mult)
                union = work.tile([P, 3 * W], BF16, tag="union")
                nc.vector.tensor_tensor(union[:], Wg, Hg, ALU.mult)
                areab3 = work.tile([P, 3 * W], BF16, tag="areab3")
                nc.vector.tensor_tensor(areab3[:, 0:W], TWp, THp, ALU.mult)
                nc.vector.tensor_copy(areab3[:, W:2 * W], areab3[:, 0:W])
                nc.vector.tensor_copy(areab3[:, 2 * W:3 * W], areab3[:, 0:W])
                nc.vector.tensor_tensor(union[:], union[:], areab3[:], ALU.add)
                nc.vector.tensor_tensor(union[:], union[:], inter[:],
                                        ALU.subtract)

                # responsible box via cross-multiplied IoU comparisons
                pq = work.tile([P, 2 * W], BF16, tag="pq")
                mk = work.tile([P, 2 * W], mybir.dt.uint16, tag="mk")
                i0, i1, i2 = (inter[:, b * W:(b + 1) * W] for b in range(3))
                u0, u1, u2 = (union[:, b * W:(b + 1) * W] for b in range(3))
                e0, e1, e2 = (enc[:, b * W:(b + 1) * W] for b in range(3))
                nc.vector.tensor_tensor(pq[:, 0:W], i1, u0, ALU.mult)
                nc.vector.tensor_tensor(pq[:, W:2 * W], i0, u1, ALU.mult)
                nc.vector.tensor_tensor(mk[:, 0:W], pq[:, 0:W], pq[:, W:2 * W],
                                        ALU.is_gt)
                mk1, mk2 = mk[:, 0:W], mk[:, W:2 * W]
                nc.vector.copy_predicated(i0, mk1, i1)
                nc.vector.copy_predicated(u0, mk1, u1)
                nc.vector.tensor_tensor(pq[:, 0:W], i2, u0, ALU.mult)
                nc.vector.tensor_tensor(pq[:, W:2 * W], i0, u2, ALU.mult)
                nc.vector.tensor_tensor(mk2, pq[:, 0:W], pq[:, W:2 * W],
                                        ALU.is_gt)
                nc.vector.copy_predicated(i0, mk2, i2)
                nc.vector.copy_predicated(u0, mk2, u2)
                nc.vector.copy_predicated(e0, mk1, e1)
                nc.vector.copy_predicated(e0, mk2, e2)

                # g = iou + union/enc  (= giou + 1)
                f32t = work.tile([P, 4 * W], F32, tag="f32t")
                uf, ef = f32t[:, 0:W], f32t[:, W:2 * W]
                ru, re = f32t[:, 2 * W:3 * W], f32t[:, 3 * W:4 * W]
                nc.vector.tensor_copy(uf, u0)
                nc.vector.tensor_copy(ef, e0)
                nc.vector.reciprocal_approx_fast(ru, uf)
                nc.vector.reciprocal_approx_fast(re, ef)
                gio = work.tile([P, 3 * W], BF16, tag="gio")
                iou, t_, g = (gio[:, b * W:(b + 1) * W] for b in range(3))
                nc.vector.tensor_tensor(iou, i0, ru, ALU.mult)
                nc.vector.tensor_tensor(t_, uf, re, ALU.mult)
                nc.vector.tensor_tensor(g, iou, t_, ALU.add)

                # conf sums and responsible-box selections
                c0 = conf[:, 0 * X + ch * W:0 * X + (ch + 1) * W]
                c1 = conf[:, 1 * X + ch * W:1 * X + (ch + 1) * W]
                c2 = conf[:, 2 * X + ch * W:2 * X + (ch + 1) * W]
                s0_0 = s0[:, 0 * X + ch * W:0 * X + (ch + 1) * W]
                s0_1 = s0[:, 1 * X + ch * W:1 * X + (ch + 1) * W]
                s0_2 = s0[:, 2 * X + ch * W:2 * X + (ch + 1) * W]

                def col(i, ch=ch):
                    j = ch * NACC + i
                    return acc[:, j:j + 1]

                # q1 = sum_b bce0_b = (c0+c1+c2) + (b0+b1+b2)
                q1 = work.tile([P, 3 * W], BF16, tag="q1")
                scr = q1[:, 2 * W:3 * W]
                scr2 = q1[:, W:2 * W]
                nc.vector.tensor_tensor(q1[:, 0:W], c0, c1, ALU.add)
                nc.vector.tensor_tensor(q1[:, 0:W], q1[:, 0:W], c2, ALU.add)
                nc.vector.tensor_tensor(q1[:, W:2 * W], s0_0, s0_1, ALU.add)
                nc.vector.tensor_tensor(q1[:, W:2 * W], q1[:, W:2 * W], s0_2,
                                        ALU.add)
                nc.vector.scalar_tensor_tensor(q1[:, 0:W], q1[:, 0:W], 0.0,
                                               q1[:, W:2 * W], ALU.add,
                                               ALU.add, accum_out=col(0))
                nc.vector.copy_predicated(c0, mk1, c1)
                nc.vector.copy_predicated(c0, mk2, c2)
                nc.vector.copy_predicated(s0_0, mk1, s0_1)
                nc.vector.copy_predicated(s0_0, mk2, s0_2)
                nc.vector.scalar_tensor_tensor(scr, q1[:, 0:W], 0.0, TC,
                                               ALU.add, ALU.mult,
                                               accum_out=col(1))
                nc.vector.scalar_tensor_tensor(scr2, s0_0, 0.0, TC,
                                               ALU.add, ALU.mult,
                                               accum_out=col(2))
                nc.vector.scalar_tensor_tensor(scr, c0, 0.0, TC,
                                               ALU.add, ALU.mult,
                                               accum_out=col(3))
                nc.vector.scalar_tensor_tensor(scr2, g, 0.0, TC,
                                               ALU.add, ALU.mult,
                                               accum_out=col(4))
                nc.vector.scalar_tensor_tensor(scr, TC, 0.0, TC,
                                               ALU.add, ALU.max,
                                               accum_out=col(5))

            nc.gpsimd.dma_start(out[:], acc[:])

    nc.compile()
    _nc_cache["nc"] = nc
    return nc


def make_in_maps(input, target):
    in_maps = []
    for c in range(CORES):
        sl = slice(c * NPC, (c + 1) * NPC)
        a = input[sl].reshape(P, X, 15)[:, :, PERM_IN].transpose(0, 2, 1)
        b = target[sl].reshape(P, X, 5)[:, :, PERM_TG].transpose(0, 2, 1)
        in_maps.append({
            "input": np.ascontiguousarray(a).astype(ml_dtypes.bfloat16).reshape(
                P, 15 * X),
            "target": np.ascontiguousarray(b).astype(ml_dtypes.bfloat16).reshape(
                P, 5 * X),
        })
    return in_maps


def kernel(input, target):
    nc = build_nc()
    in_maps = make_in_maps(input, target)
    res = run_bass_kernel_spmd(nc, in_maps, core_ids=list(range(CORES)))
    total = np.zeros(NACC, dtype=np.float64)
    for r in res.results:
        total += r["out"].reshape(P, NCH, NACC).sum(axis=(0, 1),
                                                    dtype=np.float64)
    A0, T1, NO, CR, G, NOBJ = total
    n_obj = NOBJ
    n_noobj = float(N * S * S) - n_obj
    s0r = NO + CR  # sum obj * softplus(c_resp)
    loss_noobj = (A0 - T1) / (n_noobj * NB) + (T1 - s0r) / (n_obj * (NB - 1))
    loss_obj = NO / n_obj
    loss_bbox = (2.0 * n_obj - G) / n_obj
    loss = loss_obj + loss_bbox + loss_noobj
    return (np.float32(loss), np.float32(loss_noobj), np.float32(loss_bbox),
            np.float32(loss_obj))
